# revision 7
# baseline (speedup 1.0000x reference)
"""GATv2 GNN classifier (nn_AttGNNClassifier) as an 8-core Trainium2 Bass kernel.

Strategy (graph-parallel, v3):
  - Nodes are partitioned contiguously across 8 cores; within a core they are
    degree-balance packed into NT=49 tiles of 128 (snake packing), so per-tile
    edge counts are near-uniform and padding is minimal (ragged, per-tile).
  - Two fs-table row layouts ("views"): layer 1 uses a plain core-major table
    filled by ONE AllGather; layers 2/3 use an AG-chunk-major layout with
    asymmetric chunks (16,16,13,4 tiles) so the AllGather can be issued in 4
    pieces pipelined under the previous layer's edge loop, with a tiny last
    chunk gating the next layer.
  - Edges live with their dst (core, tile), split into two sections by table
    row (< / >= 32768 so gather indices fit int16) and sorted by src row for
    HBM locality. All one-hot matrices (dst scatter in both major orders,
    graph selector) are precomputed on the host as fp8 and streamed via HWDGE.
  - The edge loop is software-pipelined 3 deep: position p issues gathers and
    one-hot loads for tile p, the z=fs+fd matmuls + leaky for tile p-1, and
    the logits/softmax/aggregation/normalize for tile p-2 — so the in-order
    engine queues always have independent work and the per-tile serial
    dependency chain is hidden.
  - h is written to DRAM and re-loaded transposed via HWDGE transpose-DMA;
    the next layer's projection and AllGather chunks are interleaved into the
    edge loop (projection in batches of 4 tiles to avoid TensorE bubbles).
  - Graph mean-pool via fp8 one-hot matmul accumulated across layer-3 tiles,
    an all-reduce of [G, 65] partials, then the tiny classifier MLP (the
    input-only pattern branch is computed during the prologue).
"""

import math
from collections import deque

import ml_dtypes
import numpy as np

import concourse.bass as bass
import concourse.bacc as bacc
import concourse.mybir as mybir
import concourse.tile as tile
from concourse import library_config
from concourse.bass_utils import run_bass_kernel_spmd

F16 = mybir.dt.float16
F32 = mybir.dt.float32
F8 = mybir.dt.float8e4
I16 = mybir.dt.int16
FP8NP = ml_dtypes.float8_e4m3

NEG_GAT = 0.2
NEG = 0.01


def _default_cfg():
    return dict(
        NC=8, N=50000, E=400000, F_IN=128, H=3, D=64, G=64, P=64, SPLIT=32768,
    )


def _derive(cfg):
    c = dict(cfg)
    c["HD"] = c["H"] * c["D"]
    c["TE"] = 256                      # table row elems (512B rows, fp16)
    c["NPC"] = 49 * 128                # nodes per core, padded
    c["NPAD"] = c["NC"] * c["NPC"]
    c["NT"] = 49
    c["NPC_REAL"] = c["N"] // c["NC"]  # 6250
    c["CHUNKS"] = [(0, 16), (16, 32), (32, 45), (45, 49)]
    c["HPAD"] = 256                    # h DRAM row elems (for transpose-DMA)
    assert c["F_IN"] <= 128
    return c


# ---------------------------------------------------------------- host prep

def _wrap16(vals, F):
    """int16 values -> [128, F] wrapped (k -> [k%16, k//16]) x8 replicated."""
    out = np.zeros((128, F), np.int16)
    k = np.arange(len(vals))
    out[k % 16, k // 16] = vals
    for g in range(1, 8):
        out[16 * g : 16 * g + 16] = out[:16]
    return out


def _pack_nodes(deg, NT):
    """Snake-pack local node indices into NT bins of <=128 by degree desc."""
    n = len(deg)
    order = np.argsort(-deg, kind="stable")
    bin_of = np.empty(n, np.int64)
    slot_of = np.empty(n, np.int64)
    counts = np.zeros(NT, np.int64)
    pos = 0
    r = 0
    while pos < n:
        take = min(NT, n - pos)
        idx = order[pos : pos + take]
        bins = np.arange(take) if r % 2 == 0 else NT - 1 - np.arange(take)
        bin_of[idx] = bins
        slot_of[idx] = counts[bins]
        counts[bins] += 1
        pos += take
        r += 1
    assert counts.max() <= 128
    return bin_of, slot_of


def _build_view(cfg, chunks, core_of, tile_of, slot_of, src, dst, e_core, e_tile,
                e_slot):
    """Build table-row mapping + edge sections/slots + idx/one-hot arrays for
    one AG chunking of the tile axis."""
    c = cfg
    NC, NT, SPLIT = c["NC"], c["NT"], c["SPLIT"]
    N = c["N"]

    chunk_of_tile = np.empty(NT, np.int64)
    tile_base = np.empty(NT, np.int64)
    chunk_base = []
    base = 0
    for ci, (t0, t1) in enumerate(chunks):
        rows_c = (t1 - t0) * 128
        chunk_base.append(base)
        for t in range(t0, t1):
            chunk_of_tile[t] = ci
            tile_base[t] = (t - t0) * 128
        base += NC * rows_c
    chunk_base = np.asarray(chunk_base)
    chunk_rows = np.asarray([(t1 - t0) * 128 for (t0, t1) in chunks])

    nid = np.arange(N)
    ci_n = chunk_of_tile[tile_of]
    row_of_node = (
        chunk_base[ci_n] + core_of * chunk_rows[ci_n] + tile_base[tile_of] + slot_of
    )

    e_srcrow = row_of_node[src]
    e_sect = (e_srcrow >= SPLIT).astype(np.int64)

    key = (e_core * NT + e_tile) * 2 + e_sect
    order = np.lexsort((e_srcrow, key))
    cnt = np.bincount(key, minlength=NC * NT * 2).reshape(NC, NT, 2)
    eca = np.maximum(1, np.ceil(cnt[:, :, 0].max(axis=0) / 128).astype(int))
    ecb = np.ceil(cnt[:, :, 1].max(axis=0) / 128).astype(int)
    KA = eca * 128
    KB = ecb * 128
    ET = KA + KB

    offA = np.concatenate([[0], np.cumsum(KA // 16)])
    offB = np.concatenate([[0], np.cumsum(KB // 16)])
    offO = np.concatenate([[0], np.cumsum(ET)])
    FA_TOT, FB_TOT, O_TOT = int(offA[-1]), int(offB[-1]), int(offO[-1])

    idxA = np.zeros((NC, 128, FA_TOT), np.int16)
    idxB = np.zeros((NC, 128, max(FB_TOT, 1)), np.int16)
    snm = np.zeros((NC, 128, O_TOT), np.float32)
    stt = np.zeros((NC, 128, O_TOT), np.float32)

    starts = np.concatenate([[0], np.cumsum(cnt.reshape(-1))]).astype(np.int64)
    for co in range(NC):
        for t in range(NT):
            for s in range(2):
                k = (co * NT + t) * 2 + s
                lo, hi = starts[k], starts[k + 1]
                e = order[lo:hi]
                n = hi - lo
                kpad = KA[t] if s == 0 else KB[t]
                assert n <= kpad, (co, t, s, n, kpad)
                base_s = 0 if s == 0 else KA[t]
                if s == 0:
                    v = np.zeros(kpad, np.int64)
                    v[:n] = e_srcrow[e]
                    idxA[co, :, offA[t] : offA[t + 1]] = _wrap16(v, kpad // 16)
                elif kpad:
                    v = np.zeros(kpad, np.int64)
                    v[:n] = e_srcrow[e] - SPLIT
                    idxB[co, :, offB[t] : offB[t + 1]] = _wrap16(v, kpad // 16)
                if n:
                    sl = base_s + np.arange(n)
                    p, j = sl % 128, sl // 128
                    snm[co, e_slot[e], offO[t] + sl] = 1.0
                    stt[co, p, offO[t] + j * 128 + e_slot[e]] = 1.0

    return dict(
        eca=eca.tolist(), ecb=ecb.tolist(), EC=(ET // 128).tolist(),
        offA=offA.tolist(), offB=offB.tolist(), offO=offO.tolist(),
        FA_TOT=FA_TOT, FB_TOT=max(FB_TOT, 1), O_TOT=O_TOT,
        chunk_base=chunk_base.tolist(), chunk_rows=chunk_rows.tolist(),
        idxA=idxA, idxB=idxB, snm=snm, stt=stt,
    )


def prep_host(inputs, cfg):
    c = cfg
    NC, N, NPC, NT, G = c["NC"], c["N"], c["NPC"], c["NT"], c["G"]
    HD, F_IN, TE = c["HD"], c["F_IN"], c["TE"]
    NPR = c["NPC_REAL"]

    src = np.asarray(inputs["src"]).astype(np.int64)
    dst = np.asarray(inputs["dst"]).astype(np.int64)
    graph_ids = np.asarray(inputs["graph_ids"]).astype(np.int64)
    x = np.asarray(inputs["inputs"]).astype(np.float32)

    core_of = np.minimum(np.arange(N) // NPR, NC - 1)
    deg = np.bincount(dst, minlength=N)
    tile_of = np.empty(N, np.int64)
    slot_of = np.empty(N, np.int64)
    for co in range(NC):
        lo, hi = co * NPR, (co + 1) * NPR
        b, s = _pack_nodes(deg[lo:hi], NT)
        tile_of[lo:hi] = b
        slot_of[lo:hi] = s

    e_core = core_of[dst]
    e_tile = tile_of[dst]
    e_slot = slot_of[dst]

    view0 = _build_view(cfg, [(0, NT)], core_of, tile_of, slot_of, src, dst,
                        e_core, e_tile, e_slot)
    view1 = _build_view(cfg, c["CHUNKS"], core_of, tile_of, slot_of, src, dst,
                        e_core, e_tile, e_slot)

    gsel = np.zeros((NC, 128, NT * G), np.float32)
    nid = np.arange(N)
    gsel[core_of, slot_of, tile_of * G + graph_ids[nid]] = 1.0

    x_fm = np.zeros((NC, F_IN, NPC), np.float16)
    colv = tile_of * 128 + slot_of
    for co in range(NC):
        m = core_of == co
        x_fm[co][:, colv[m]] = x[m].T.astype(np.float16)

    rep = lambda v, p=128: np.broadcast_to(
        np.asarray(v, np.float16)[None, :], (p, len(v))
    ).copy()

    def w16(k):
        return np.asarray(inputs[k]).astype(np.float16)

    def ws_pad(k):
        w = np.asarray(inputs[k]).astype(np.float16)
        out = np.zeros((w.shape[0], TE), np.float16)
        out[:, :HD] = w
        return out

    a_flat = [np.asarray(inputs[f"a{l}"]).astype(np.float32).reshape(-1) for l in (1, 2, 3)]
    b_flat = [np.asarray(inputs[f"b{l}"]).astype(np.float32) for l in (1, 2, 3)]
    b3m = b_flat[2].reshape(c["H"], c["D"]).mean(0)

    bex = np.asarray(inputs["bex"]).astype(np.float32)
    bex96 = np.concatenate([bex, bex, bex])

    common = dict(
        W1s=ws_pad("W1s"), W1d=w16("W1d"),
        W2s=ws_pad("W2s"), W2d=w16("W2d"),
        W3s=ws_pad("W3s"), W3d=w16("W3d"),
        a1_rep=rep(a_flat[0]), a2_rep=rep(a_flat[1]), a3_rep=rep(a_flat[2]),
        b1_rep=rep(b_flat[0]), b2_rep=rep(b_flat[1]),
        b3m_rep=rep(b3m),
        ident8=np.eye(128, dtype=np.float32).astype(FP8NP),
        ident=np.eye(128, dtype=np.float16),
        p1T=w16("p1").T.copy(), p2T=w16("p2").T.copy(), p3T=w16("p3").T.copy(),
        Wex=w16("Wex"), bex96_rep=rep(bex96, G),
        Wpat=w16("Wpat"), bpat_rep=rep(np.asarray(inputs["bpat"], np.float32), G),
        Wc1=w16("Wc1"), bc1_rep=rep(np.asarray(inputs["bc1"], np.float32), G),
        Wc2=w16("Wc2"), bc2_rep=rep(np.asarray(inputs["bc2"], np.float32), G),
        Wc3=w16("Wc3"), bc3_rep=rep(np.asarray(inputs["bc3"], np.float32), G),
    )

    in_maps = []
    for co in range(NC):
        m = dict(common)
        m["x_fm"] = x_fm[co]
        m["gsel_all"] = gsel[co].astype(FP8NP)
        for vi, v in ((0, view0), (1, view1)):
            m[f"idxA{vi}"] = v["idxA"][co]
            m[f"idxB{vi}"] = v["idxB"][co]
            m[f"snm{vi}"] = v["snm"][co].astype(FP8NP)
            m[f"st{vi}"] = v["stt"][co].astype(FP8NP)
        in_maps.append(m)

    meta = dict(views=[
        {k: v[k] for k in ("eca", "ecb", "EC", "offA", "offB", "offO",
                           "FA_TOT", "FB_TOT", "O_TOT", "chunk_base",
                           "chunk_rows")}
        for v in (view0, view1)
    ])
    return in_maps, meta


# ---------------------------------------------------------------- device build

def build_gat(cfg, meta):
    c = cfg
    NC, NPC, NPAD, NT, G = c["NC"], c["NPC"], c["NPAD"], c["NT"], c["G"]
    H, D, HD, F_IN, TE, SPLIT = c["H"], c["D"], c["HD"], c["F_IN"], c["TE"], c["SPLIT"]
    HPAD = c["HPAD"]
    CHUNKS = c["CHUNKS"]
    V = meta["views"]
    ECMAX = max(max(V[0]["EC"]), max(V[1]["EC"]))
    view_of = {1: 0, 2: 1, 3: 1}

    nc = bacc.Bacc("TRN2", target_bir_lowering=False, debug=False, num_devices=NC,
                   num_swdge_queues=4)

    def din(name, shape, dt=F16):
        return nc.dram_tensor(name, shape, dt, kind="ExternalInput")

    x_fm = din("x_fm", [F_IN, NPC])
    gsel_all = din("gsel_all", [128, NT * G], F8)
    idx_d = {}
    oh_d = {}
    for vi in (0, 1):
        idx_d[vi] = (
            din(f"idxA{vi}", [128, V[vi]["FA_TOT"]], I16),
            din(f"idxB{vi}", [128, V[vi]["FB_TOT"]], I16),
        )
        oh_d[vi] = (
            din(f"snm{vi}", [128, V[vi]["O_TOT"]], F8),
            din(f"st{vi}", [128, V[vi]["O_TOT"]], F8),
        )

    Wmat = {
        1: (din("W1s", [F_IN, TE]), din("W1d", [F_IN, HD])),
        2: (din("W2s", [HD, TE]), din("W2d", [HD, HD])),
        3: (din("W3s", [HD, TE]), din("W3d", [HD, HD])),
    }
    a_rep = {l: din(f"a{l}_rep", [128, HD]) for l in (1, 2, 3)}
    b_rep = {1: din("b1_rep", [128, HD]), 2: din("b2_rep", [128, HD])}
    b3m_rep = din("b3m_rep", [128, D])
    ident8_d = din("ident8", [128, 128], F8)
    ident_d = din("ident", [128, 128])
    p123T = [din("p1T", [64, G]), din("p2T", [64, G]), din("p3T", [64, G])]
    Wex = din("Wex", [64, 32])
    bex96_rep = din("bex96_rep", [G, 96])
    Wpat = din("Wpat", [96, 64])
    bpat_rep = din("bpat_rep", [G, 64])
    Wc1 = din("Wc1", [128, 64])
    bc1_rep = din("bc1_rep", [G, 64])
    Wc2 = din("Wc2", [64, 32])
    bc2_rep = din("bc2_rep", [G, 32])
    Wc3 = din("Wc3", [32, 2])
    bc3_rep = din("bc3_rep", [G, 2])

    out = nc.dram_tensor("out", [G, 2], F32, kind="ExternalOutput")

    # internal DRAM
    fs_own1 = nc.dram_tensor("fs_own1", [NPC, TE], F16)
    fs_own = {
        (l, ci): nc.dram_tensor(f"fs_own{l}_{ci}", [rows, TE], F16)
        for l in (2, 3)
        for ci, rows in enumerate(V[1]["chunk_rows"])
    }
    h_dram = {
        (l, ci): nc.dram_tensor(f"h{l}_{ci}", [V[1]["chunk_rows"][ci], HPAD], F16)
        for l in (1, 2)
        for ci in range(len(CHUNKS))
    }
    fs_full = {
        l: nc.dram_tensor(f"fs_full{l}", [NPAD, TE], F16, addr_space="Shared")
        for l in (1, 2, 3)
    }
    partials = nc.dram_tensor("partials", [G, 65], F32)
    partials_red = nc.dram_tensor("partials_red", [G, 65], F32, addr_space="Shared")

    groups = [list(range(NC))]
    FCH = [(0, 128), (128, 64)]

    with tile.TileContext(nc) as tc:
        with (
            tc.tile_pool(name="const", bufs=1) as cpool,
            tc.tile_pool(name="wpool", bufs=1) as wpool,
            tc.tile_pool(name="hT", bufs=2) as hTpool,
            tc.tile_pool(name="proj", bufs=3) as ppool,
            tc.tile_pool(name="edge", bufs=2) as epool,
            tc.tile_pool(name="gath", bufs=3) as gpool,
            tc.tile_pool(name="oneh", bufs=3) as opool,
            tc.tile_pool(name="small", bufs=2) as spool,
            tc.tile_pool(name="psA", bufs=2, space="PSUM") as psA,
            tc.tile_pool(name="psZ", bufs=2, space="PSUM") as psZ,
            tc.tile_pool(name="psB", bufs=2, space="PSUM") as psB,
            tc.tile_pool(name="psT", bufs=1, space="PSUM") as psT,
            tc.tile_pool(name="psG", bufs=1, space="PSUM") as psG,
        ):
            nc.gpsimd.load_library(library_config.mlp)

            # ---------- resident constants
            ident8_t = cpool.tile([128, 128], F8)
            nc.sync.dma_start(ident8_t[:], ident8_d[:])
            ident_t = cpool.tile([128, 128], F16)
            nc.sync.dma_start(ident_t[:], ident_d[:])
            a_t = {l: cpool.tile([128, HD], F16, tag=f"a{l}", name=f"a{l}_t") for l in (1, 2, 3)}
            for l in (1, 2, 3):
                nc.sync.dma_start(a_t[l][:], a_rep[l][:])
            b_t = {l: cpool.tile([128, HD], F16, tag=f"b{l}", name=f"b{l}_t") for l in (1, 2)}
            for l in (1, 2):
                nc.sync.dma_start(b_t[l][:], b_rep[l][:])
            b3m_t = cpool.tile([128, D], F16)
            nc.sync.dma_start(b3m_t[:], b3m_rep[:])
            x_fm_t = cpool.tile([F_IN, NPC], F16)
            nc.sync.dma_start(x_fm_t[:], x_fm[:])
            gsel_t = cpool.tile([128, NT * G], F8)
            nc.sync.dma_start(gsel_t[:], gsel_all[:])
            idx_t = {}
            for vi in (0, 1):
                ta = cpool.tile([128, V[vi]["FA_TOT"]], I16, tag=f"ixA{vi}", name=f"idxA{vi}_t")
                nc.sync.dma_start(ta[:], idx_d[vi][0][:])
                tb = cpool.tile([128, V[vi]["FB_TOT"]], I16, tag=f"ixB{vi}", name=f"idxB{vi}_t")
                nc.sync.dma_start(tb[:], idx_d[vi][1][:])
                idx_t[vi] = (ta, tb)

            Wt = {}
            for l in (1, 2, 3):
                kdim = F_IN if l == 1 else HD
                chs = [(0, kdim)] if kdim <= 128 else FCH
                Wt[l] = []
                for k, (off, sz) in enumerate(chs):
                    ws = wpool.tile([sz, TE], F16, tag=f"W{l}s{k}", name=f"W{l}s{k}_t")
                    wd = wpool.tile([sz, HD], F16, tag=f"W{l}d{k}", name=f"W{l}d{k}_t")
                    nc.sync.dma_start(ws[:], Wmat[l][0][off : off + sz, :])
                    nc.sync.dma_start(wd[:], Wmat[l][1][off : off + sz, :])
                    Wt[l].append((ws, wd))

            fd_res = [
                cpool.tile([128, NT, HD], F16, tag=f"fd{i}", name=f"fd_res{i}")
                for i in (0, 1)
            ]
            fd_of = {1: fd_res[0], 2: fd_res[1], 3: fd_res[0]}

            gp_ps = psG.tile([G, 65], F32, space="PSUM")

            # ---------- pattern branch early (input-only)
            px_ps = psA.tile([G, 96], F32, space="PSUM", tag="psP", name="px_ps")
            Wex_t = spool.tile([64, 32], F16, tag="Wex_t")
            nc.sync.dma_start(Wex_t[:], Wex[:])
            for i in range(3):
                pT = spool.tile([64, G], F16, tag=f"pT{i}", name=f"pT{i}")
                nc.sync.dma_start(pT[:], p123T[i][:])
                nc.tensor.matmul(
                    px_ps[:, 32 * i : 32 * i + 32], lhsT=pT[:], rhs=Wex_t[:],
                    start=True, stop=True,
                )
            bex_t = spool.tile([G, 96], F16, tag="bex_t")
            nc.sync.dma_start(bex_t[:], bex96_rep[:])
            pxc = spool.tile([G, 96], F16, tag="pxc")
            nc.vector.tensor_tensor(
                out=pxc[:], in0=px_ps[:], in1=bex_t[:], op=mybir.AluOpType.add
            )

            # ---------- helpers
            def proj_tile(l, lhs_chunks, t, fs_dst, row0):
                """Project tile t for layer l into fs_dst rows [row0:row0+128]
                and fd_of[l][:, t, :]."""
                ps_fs = psA.tile([128, TE], F32, space="PSUM", tag="psP", name="ps_fs")
                ps_fd = psA.tile([128, HD], F32, space="PSUM", tag="psP", name="ps_fd")
                for k, lt in enumerate(lhs_chunks):
                    nc.tensor.matmul(
                        ps_fs[:], lhsT=lt, rhs=Wt[l][k][0][:],
                        start=(k == 0), stop=(k == len(lhs_chunks) - 1),
                    )
                for k, lt in enumerate(lhs_chunks):
                    nc.tensor.matmul(
                        ps_fd[:], lhsT=lt, rhs=Wt[l][k][1][:],
                        start=(k == 0), stop=(k == len(lhs_chunks) - 1),
                    )
                fs_sb = ppool.tile([128, TE], F16, tag="fs_sb")
                nc.scalar.copy(fs_sb[:], ps_fs[:])
                nc.scalar.copy(fd_of[l][:, t, :], ps_fd[:])
                nc.sync.dma_start(fs_dst[row0 : row0 + 128, :], fs_sb[:])

            def ag_full_l1():
                nc.gpsimd.collective_compute(
                    "AllGather",
                    mybir.AluOpType.bypass,
                    replica_groups=groups,
                    ins=[fs_own1[:].rearrange("a b -> (a b)")],
                    outs=[fs_full[1][:].rearrange("a b -> (a b)")],
                )

            def ag_chunk(l, ci):
                rows = V[1]["chunk_rows"][ci]
                base = V[1]["chunk_base"][ci]
                nc.gpsimd.collective_compute(
                    "AllGather",
                    mybir.AluOpType.bypass,
                    replica_groups=groups,
                    ins=[fs_own[(l, ci)][:].rearrange("a b -> (a b)")],
                    outs=[
                        fs_full[l][base : base + NC * rows, :].rearrange(
                            "a b -> (a b)"
                        )
                    ],
                )

            # ---------- layer-1 projection prologue: all tiles, one AG
            for t in range(NT):
                proj_tile(1, [x_fm_t[:, bass.ts(t, 128)]], t, fs_own1, t * 128)
            ag_full_l1()

            # ---------- edge loop state (per-tile pipeline stages)
            def prefetch(l, t):
                vi = view_of[l]
                v = V[vi]
                ea, eb = v["eca"][t], v["ecb"][t]
                ec = v["EC"][t]
                A = gpool.tile([128, ECMAX, TE], F16, tag="A")
                nc.gpsimd.dma_gather(
                    out_ap=A[:, :ea, :],
                    in_ap=fs_full[l][:SPLIT, :],
                    idxs_ap=idx_t[vi][0][:, v["offA"][t] : v["offA"][t + 1]],
                    num_idxs=ea * 128,
                    num_idxs_reg=ea * 128,
                    elem_size=TE,
                    queue_num=(2 * t) % 4,
                )
                if eb:
                    nc.gpsimd.dma_gather(
                        out_ap=A[:, ea : ea + eb, :],
                        in_ap=fs_full[l][SPLIT:, :],
                        idxs_ap=idx_t[vi][1][:, v["offB"][t] : v["offB"][t + 1]],
                        num_idxs=eb * 128,
                        num_idxs_reg=eb * 128,
                        elem_size=TE,
                        queue_num=(2 * t + 1) % 4,
                    )
                ET_t = ec * 128
                snm_t = opool.tile([128, ECMAX * 128], F8, tag="snm")
                nc.sync.dma_start(
                    snm_t[:, :ET_t], oh_d[vi][0][:, v["offO"][t] : v["offO"][t + 1]]
                )
                st_t = opool.tile([128, ECMAX * 128], F8, tag="st")
                nc.sync.dma_start(
                    st_t[:, :ET_t], oh_d[vi][1][:, v["offO"][t] : v["offO"][t + 1]]
                )
                return dict(A=A, snm=snm_t, st=st_t, ec=ec)

            def stage1(l, t, s):
                """z = fd[dst] + fs in PSUM chunk-pairs; leaky -> C."""
                ec, A, snm_t = s["ec"], s["A"], s["snm"]
                C = epool.tile([128, ECMAX, HD], F16, tag="C")
                for j0 in range(0, ec, 2):
                    jn = min(2, ec - j0)
                    zps = psZ.tile([128, 2, HD], F32, space="PSUM", tag="zps", name="zps")
                    for j in range(j0, j0 + jn):
                        nc.tensor.matmul(
                            zps[:, j - j0, :],
                            lhsT=snm_t[:, bass.ts(j, 128)],
                            rhs=fd_of[l][:, t, :],
                            start=True, stop=False,
                        )
                        nc.tensor.matmul(
                            zps[:, j - j0, :],
                            lhsT=ident8_t[:],
                            rhs=A[:, j, :HD],
                            start=False, stop=True,
                        )
                    nc.scalar.activation(
                        C[:, j0 : j0 + jn, :],
                        zps[:, :jn, :],
                        mybir.ActivationFunctionType.Prelu,
                        alpha=NEG_GAT,
                    )
                s["C"] = C

            def stage2(l, t, s, ci):
                ec, A, st_t, C = s["ec"], s["A"], s["st"], s["C"]
                AM = epool.tile([128, ECMAX, HD], F16, tag="AM")
                nc.vector.tensor_tensor(
                    out=AM[:, :ec, :], in0=C[:, :ec, :],
                    in1=a_t[l][:, None, :].to_broadcast([128, ec, HD]),
                    op=mybir.AluOpType.mult,
                )
                logit = spool.tile([128, ECMAX, H], F32, tag="logit")
                nc.vector.tensor_reduce(
                    out=logit[:, :ec, :],
                    in_=AM[:, :ec, :].rearrange("p a (h d) -> p a h d", h=H),
                    axis=mybir.AxisListType.X,
                    op=mybir.AluOpType.add,
                )
                EFX = epool.tile([128, ECMAX, HD + H], F16, tag="EFX")
                ex = EFX[:, :ec, HD : HD + H]
                nc.scalar.activation(
                    ex, logit[:, :ec, :], mybir.ActivationFunctionType.Exp
                )
                nc.vector.tensor_tensor(
                    out=EFX[:, :ec, :HD].rearrange("p a (h d) -> p a h d", h=H),
                    in0=A[:, :ec, :HD].rearrange("p a (h d) -> p a h d", h=H),
                    in1=ex[:, :, :, None].to_broadcast([128, ec, H, D]),
                    op=mybir.AluOpType.mult,
                )
                ps_ud = psB.tile([128, HD + H], F32, space="PSUM", tag="ps_ud", name="ps_ud")
                for j in range(ec):
                    nc.tensor.matmul(
                        ps_ud[:], lhsT=st_t[:, bass.ts(j, 128)],
                        rhs=EFX[:, j, :],
                        start=(j == 0), stop=(j == ec - 1),
                    )
                dmax = spool.tile([128, H], F32, tag="dmax")
                nc.vector.tensor_scalar_max(dmax[:], ps_ud[:, HD : HD + H], 1e-9)
                rden = spool.tile([128, H], F32, tag="rden")
                nc.vector.reciprocal(rden[:], dmax[:])
                hm = spool.tile([128, H, D], F16, tag="hm")
                nc.vector.tensor_tensor(
                    out=hm[:],
                    in0=ps_ud[:, :HD].rearrange("p (h d) -> p h d", h=H),
                    in1=rden[:, :, None].to_broadcast([128, H, D]),
                    op=mybir.AluOpType.mult,
                )
                if l < 3:
                    t0c = CHUNKS[ci][0]
                    h_sb = ppool.tile([128, HPAD], F16, tag="h_sb")
                    nc.vector.tensor_tensor(
                        out=h_sb[:, :HD],
                        in0=hm[:].rearrange("p h d -> p (h d)"),
                        in1=b_t[l][:],
                        op=mybir.AluOpType.add,
                    )
                    nc.vector.memset(h_sb[:, HD:], 0.0)
                    nc.sync.dma_start(
                        h_dram[(l, ci)][bass.ts(t - t0c, 128), :], h_sb[:]
                    )
                else:
                    rhs65 = ppool.tile([128, 65], F16, tag="rhs65")
                    t01 = spool.tile([128, D], F16, tag="t01")
                    nc.vector.tensor_tensor(
                        out=t01[:], in0=hm[:, 0, :], in1=hm[:, 1, :],
                        op=mybir.AluOpType.add,
                    )
                    t012 = spool.tile([128, D], F16, tag="t012")
                    nc.vector.tensor_tensor(
                        out=t012[:], in0=t01[:], in1=hm[:, 2, :],
                        op=mybir.AluOpType.add,
                    )
                    nc.vector.scalar_tensor_tensor(
                        out=rhs65[:, :D], in0=t012[:], scalar=1.0 / H,
                        in1=b3m_t[:], op0=mybir.AluOpType.mult,
                        op1=mybir.AluOpType.add,
                    )
                    nc.vector.memset(rhs65[:, 64:65], 1.0)
                    nc.tensor.matmul(
                        gp_ps[:], lhsT=gsel_t[:, bass.ts(t, G)], rhs=rhs65[:],
                        start=(t == 0), stop=(t == NT - 1),
                    )

            chunk_of = {}
            for ci, (t0, t1) in enumerate(CHUNKS):
                for t in range(t0, t1):
                    chunk_of[t] = ci
            chunk_last = {CHUNKS[ci][1] - 1: ci for ci in range(len(CHUNKS))}

            # ---------- layers: pipelined edge loop
            for l in (1, 2, 3):
                state = {}
                pending = deque()
                pending_ag = deque()

                def drain_proj(n):
                    for _ in range(min(n, len(pending))):
                        ll, ci, tg, hT1, hT2 = pending.popleft()
                        tloc = tg - CHUNKS[ci][0]
                        proj_tile(
                            ll,
                            [hT1[:, bass.ts(tloc, 128)], hT2[:64, bass.ts(tloc, 128)]],
                            tg, fs_own[(ll, ci)], tloc * 128,
                        )
                    if not pending and pending_ag:
                        ll, ci = pending_ag.popleft()
                        ag_chunk(ll, ci)

                for p in range(NT + 2):
                    if p < NT:
                        state[p] = prefetch(l, p)
                    if 1 <= p <= NT:
                        stage1(l, p - 1, state[p - 1])
                    if p >= 2:
                        t2 = p - 2
                        stage2(l, t2, state[t2], chunk_of[t2])
                        del state[t2]["C"]
                        if l < 3 and t2 in chunk_last:
                            ci = chunk_last[t2]
                            rows = V[1]["chunk_rows"][ci]
                            hT1 = hTpool.tile([128, 2048], F16, tag="hT1", name="hT1")
                            nc.sync.dma_start(
                                hT1[:, :rows], h_dram[(l, ci)][:, 0:128],
                                transpose=True,
                            )
                            hT2 = hTpool.tile([128, 2048], F16, tag="hT2", name="hT2")
                            nc.sync.dma_start(
                                hT2[:, :rows], h_dram[(l, ci)][:, 128:256],
                                transpose=True,
                            )
                            for tg in range(CHUNKS[ci][0], CHUNKS[ci][1]):
                                pending.append((l + 1, ci, tg, hT1, hT2))
                            pending_ag.append((l + 1, ci))
                    drain_proj(4)
                while pending or pending_ag:
                    drain_proj(4)

            # ================= epilogue
            part_sb = spool.tile([G, 65], F32, tag="part_sb")
            nc.vector.tensor_copy(part_sb[:], gp_ps[:])
            nc.sync.dma_start(partials[:], part_sb[:])
            nc.gpsimd.collective_compute(
                "AllReduce",
                mybir.AluOpType.add,
                replica_groups=groups,
                ins=[partials[:]],
                outs=[partials_red[:]],
            )
            red_sb = spool.tile([G, 65], F32, tag="red_sb")
            nc.sync.dma_start(red_sb[:], partials_red[:])

            xg = spool.tile([G, 128], F16, tag="xg")
            rc = spool.tile([G, 1], F32, tag="rc")
            cnt1 = spool.tile([G, 1], F32, tag="cnt1")
            nc.vector.tensor_scalar_max(cnt1[:], red_sb[:, 64:65], 1.0)
            nc.vector.reciprocal(rc[:], cnt1[:])
            nc.vector.tensor_tensor(
                out=xg[:, :64], in0=red_sb[:, :64],
                in1=rc[:].to_broadcast([G, 64]), op=mybir.AluOpType.mult,
            )

            def small_mm(x_sb, pdim, w_t, b_t_, odim, leaky, out_ap, out_f32=False):
                tp = psT.tile([128, 128], F16, space="PSUM", tag="tp", name="ep_tp")
                nc.tensor.transpose(tp[:pdim, :G], x_sb[:, :pdim], ident_t[:G, :G])
                xT = spool.tile([128, G], F16, tag="ep_xT")
                nc.scalar.copy(xT[:pdim, :], tp[:pdim, :G])
                mm = psA.tile([G, 64], F32, space="PSUM", tag="psP", name="ep_mm")
                nc.tensor.matmul(
                    mm[:, :odim], lhsT=xT[:pdim, :], rhs=w_t[:], start=True, stop=True
                )
                tmp = spool.tile([G, 64], F32 if out_f32 else F16, tag="ep_tmp")
                nc.vector.tensor_tensor(
                    out=tmp[:, :odim], in0=mm[:, :odim], in1=b_t_[:],
                    op=mybir.AluOpType.add,
                )
                if leaky:
                    nc.vector.scalar_tensor_tensor(
                        out=out_ap, in0=tmp[:, :odim], scalar=NEG,
                        in1=tmp[:, :odim], op0=mybir.AluOpType.mult,
                        op1=mybir.AluOpType.max,
                    )
                else:
                    nc.vector.tensor_copy(out_ap, tmp[:, :odim])

            Wpat_t = spool.tile([96, 64], F16, tag="Wpat_t")
            nc.sync.dma_start(Wpat_t[:], Wpat[:])
            bpat_t = spool.tile([G, 64], F16, tag="bpat_t")
            nc.sync.dma_start(bpat_t[:], bpat_rep[:])
            small_mm(pxc, 96, Wpat_t, bpat_t, 64, True, xg[:, 64:128])

            Wc1_t = spool.tile([128, 64], F16, tag="Wc1_t")
            nc.sync.dma_start(Wc1_t[:], Wc1[:])
            bc1_t = spool.tile([G, 64], F16, tag="bc1_t")
            nc.sync.dma_start(bc1_t[:], bc1_rep[:])
            h1 = spool.tile([G, 64], F16, tag="ep_h1")
            small_mm(xg, 128, Wc1_t, bc1_t, 64, True, h1[:])

            Wc2_t = spool.tile([64, 32], F16, tag="Wc2_t")
            nc.sync.dma_start(Wc2_t[:], Wc2[:])
            bc2_t = spool.tile([G, 32], F16, tag="bc2_t")
            nc.sync.dma_start(bc2_t[:], bc2_rep[:])
            h2 = spool.tile([G, 32], F16, tag="ep_h2")
            small_mm(h1, 64, Wc2_t, bc2_t, 32, True, h2[:])

            Wc3_t = spool.tile([32, 2], F16, tag="Wc3_t")
            nc.sync.dma_start(Wc3_t[:], Wc3[:])
            bc3_t = spool.tile([G, 2], F16, tag="bc3_t")
            nc.sync.dma_start(bc3_t[:], bc3_rep[:])
            h3 = spool.tile([G, 2], F32, tag="ep_h3")
            small_mm(h2, 32, Wc3_t, bc3_t, 2, False, h3[:], out_f32=True)
            nc.sync.dma_start(out[:], h3[:])

    nc.finalize()
    return nc


# ---------------------------------------------------------------- entry point

def _run(inputs, trace=False, **trace_kwargs):
    cfg = _derive(_default_cfg())
    in_maps, meta = prep_host(inputs, cfg)
    nc = build_gat(cfg, meta)
    res = run_bass_kernel_spmd(
        nc, in_maps, core_ids=list(range(cfg["NC"])), trace=trace, **trace_kwargs
    )
    return np.asarray(res.results[0]["out"], np.float32), res


def kernel(**inputs):
    out, _ = _run(inputs, trace=False)
    return out


# revision 17
# speedup vs baseline: 1.3490x; 1.3490x over previous
"""GATv2 GNN classifier (nn_AttGNNClassifier) as an 8-core Trainium2 Bass kernel.

Strategy (graph-parallel, v3):
  - Nodes are partitioned contiguously across 8 cores; within a core they are
    degree-balance packed into NT=49 tiles of 128 (snake packing), so per-tile
    edge counts are near-uniform and padding is minimal (ragged, per-tile).
  - Two fs-table row layouts ("views"): layer 1 uses a plain core-major table
    filled by ONE AllGather; layers 2/3 use an AG-chunk-major layout with
    asymmetric chunks (16,16,13,4 tiles) so the AllGather can be issued in 4
    pieces pipelined under the previous layer's edge loop, with a tiny last
    chunk gating the next layer.
  - Edges live with their dst (core, tile), split into two sections by table
    row (< / >= 32768 so gather indices fit int16) and sorted by src row for
    HBM locality. All one-hot matrices (dst scatter in both major orders,
    graph selector) are precomputed on the host as fp8 and streamed via HWDGE.
  - The edge loop is software-pipelined 3 deep: position p issues gathers and
    one-hot loads for tile p, the z=fs+fd matmuls + leaky for tile p-1, and
    the logits/softmax/aggregation/normalize for tile p-2 — so the in-order
    engine queues always have independent work and the per-tile serial
    dependency chain is hidden.
  - h is written to DRAM and re-loaded transposed via HWDGE transpose-DMA;
    the next layer's projection and AllGather chunks are interleaved into the
    edge loop (projection in batches of 4 tiles to avoid TensorE bubbles).
  - Graph mean-pool via fp8 one-hot matmul accumulated across layer-3 tiles,
    an all-reduce of [G, 65] partials, then the tiny classifier MLP (the
    input-only pattern branch is computed during the prologue).
"""

import math
from collections import deque

import ml_dtypes
import numpy as np

import concourse.bass as bass
import concourse.bacc as bacc
import concourse.mybir as mybir
import concourse.tile as tile
from concourse import library_config
from concourse.bass_utils import run_bass_kernel_spmd

F16 = mybir.dt.float16
F32 = mybir.dt.float32
F8 = mybir.dt.float8e4
I16 = mybir.dt.int16
FP8NP = ml_dtypes.float8_e4m3

NEG_GAT = 0.2
NEG = 0.01
GSZ = 2  # tiles per gather group


def _default_cfg():
    return dict(
        NC=8, N=50000, E=400000, F_IN=128, H=3, D=64, G=64, P=64, SPLIT=32768,
    )


def _derive(cfg):
    c = dict(cfg)
    c["HD"] = c["H"] * c["D"]
    c["TE"] = 256                      # table row elems (512B rows, fp16)
    c["NPC"] = 49 * 128                # nodes per core, padded
    c["NPAD"] = c["NC"] * c["NPC"]
    c["NT"] = 49
    c["NPC_REAL"] = c["N"] // c["NC"]  # 6250
    c["CHUNKS"] = [(0, 16), (16, 32), (32, 45), (45, 49)]
    c["HPAD"] = 256                    # h DRAM row elems (for transpose-DMA)
    assert c["F_IN"] <= 128
    return c


# ---------------------------------------------------------------- host prep

def _wrap16(vals, F):
    """int16 values -> [128, F] wrapped (k -> [k%16, k//16]) x8 replicated."""
    out = np.zeros((128, F), np.int16)
    k = np.arange(len(vals))
    out[k % 16, k // 16] = vals
    for g in range(1, 8):
        out[16 * g : 16 * g + 16] = out[:16]
    return out


def _pack_nodes(deg, NT):
    """Snake-pack local node indices into NT bins of <=128 by degree desc."""
    n = len(deg)
    order = np.argsort(-deg, kind="stable")
    bin_of = np.empty(n, np.int64)
    slot_of = np.empty(n, np.int64)
    counts = np.zeros(NT, np.int64)
    pos = 0
    r = 0
    while pos < n:
        take = min(NT, n - pos)
        idx = order[pos : pos + take]
        bins = np.arange(take) if r % 2 == 0 else NT - 1 - np.arange(take)
        bin_of[idx] = bins
        slot_of[idx] = counts[bins]
        counts[bins] += 1
        pos += take
        r += 1
    assert counts.max() <= 128
    return bin_of, slot_of


def _build_view(cfg, chunks, core_of, tile_of, slot_of, src, dst, e_core, e_tile,
                e_slot):
    """Build table-row mapping + edge sections/slots + idx/one-hot arrays for
    one AG chunking of the tile axis."""
    c = cfg
    NC, NT, SPLIT = c["NC"], c["NT"], c["SPLIT"]
    N = c["N"]

    chunk_of_tile = np.empty(NT, np.int64)
    tile_base = np.empty(NT, np.int64)
    chunk_base = []
    base = 0
    for ci, (t0, t1) in enumerate(chunks):
        rows_c = (t1 - t0) * 128
        chunk_base.append(base)
        for t in range(t0, t1):
            chunk_of_tile[t] = ci
            tile_base[t] = (t - t0) * 128
        base += NC * rows_c
    chunk_base = np.asarray(chunk_base)
    chunk_rows = np.asarray([(t1 - t0) * 128 for (t0, t1) in chunks])

    nid = np.arange(N)
    ci_n = chunk_of_tile[tile_of]
    row_of_node = (
        chunk_base[ci_n] + core_of * chunk_rows[ci_n] + tile_base[tile_of] + slot_of
    )

    e_srcrow = row_of_node[src]
    e_sect = (e_srcrow >= SPLIT).astype(np.int64)

    key = (e_core * NT + e_tile) * 2 + e_sect
    order = np.lexsort((e_srcrow, key))
    cnt = np.bincount(key, minlength=NC * NT * 2).reshape(NC, NT, 2)
    eca = np.maximum(1, np.ceil(cnt[:, :, 0].max(axis=0) / 128).astype(int))
    ecb = np.ceil(cnt[:, :, 1].max(axis=0) / 128).astype(int)
    KA = eca * 128
    KB = ecb * 128
    ET = KA + KB

    offA = np.concatenate([[0], np.cumsum(KA // 16)])
    offB = np.concatenate([[0], np.cumsum(KB // 16)])
    offO = np.concatenate([[0], np.cumsum(ET)])
    FA_TOT, FB_TOT, O_TOT = int(offA[-1]), int(offB[-1]), int(offO[-1])

    idxA = np.zeros((NC, 128, FA_TOT), np.int16)
    idxB = np.zeros((NC, 128, max(FB_TOT, 1)), np.int16)
    snm = np.zeros((NC, 128, O_TOT), np.float32)
    stt = np.zeros((NC, 128, O_TOT), np.float32)

    starts = np.concatenate([[0], np.cumsum(cnt.reshape(-1))]).astype(np.int64)
    for co in range(NC):
        for t in range(NT):
            for s in range(2):
                k = (co * NT + t) * 2 + s
                lo, hi = starts[k], starts[k + 1]
                e = order[lo:hi]
                n = hi - lo
                kpad = KA[t] if s == 0 else KB[t]
                assert n <= kpad, (co, t, s, n, kpad)
                base_s = 0 if s == 0 else KA[t]
                if s == 0:
                    v = np.zeros(kpad, np.int64)
                    v[:n] = e_srcrow[e]
                    idxA[co, :, offA[t] : offA[t + 1]] = _wrap16(v, kpad // 16)
                elif kpad:
                    v = np.zeros(kpad, np.int64)
                    v[:n] = e_srcrow[e] - SPLIT
                    idxB[co, :, offB[t] : offB[t + 1]] = _wrap16(v, kpad // 16)
                if n:
                    sl = base_s + np.arange(n)
                    p, j = sl % 128, sl // 128
                    snm[co, e_slot[e], offO[t] + sl] = 1.0
                    stt[co, p, offO[t] + j * 128 + e_slot[e]] = 1.0

    return dict(
        eca=eca.tolist(), ecb=ecb.tolist(), EC=(ET // 128).tolist(),
        offA=offA.tolist(), offB=offB.tolist(), offO=offO.tolist(),
        FA_TOT=FA_TOT, FB_TOT=max(FB_TOT, 1), O_TOT=O_TOT,
        chunk_base=chunk_base.tolist(), chunk_rows=chunk_rows.tolist(),
        idxA=idxA, idxB=idxB, snm=snm, stt=stt,
    )


def prep_host(inputs, cfg):
    c = cfg
    NC, N, NPC, NT, G = c["NC"], c["N"], c["NPC"], c["NT"], c["G"]
    HD, F_IN, TE = c["HD"], c["F_IN"], c["TE"]
    NPR = c["NPC_REAL"]

    src = np.asarray(inputs["src"]).astype(np.int64)
    dst = np.asarray(inputs["dst"]).astype(np.int64)
    graph_ids = np.asarray(inputs["graph_ids"]).astype(np.int64)
    x = np.asarray(inputs["inputs"]).astype(np.float32)

    core_of = np.minimum(np.arange(N) // NPR, NC - 1)
    deg = np.bincount(dst, minlength=N)
    tile_of = np.empty(N, np.int64)
    slot_of = np.empty(N, np.int64)
    for co in range(NC):
        lo, hi = co * NPR, (co + 1) * NPR
        b, s = _pack_nodes(deg[lo:hi], NT)
        tile_of[lo:hi] = b
        slot_of[lo:hi] = s

    e_core = core_of[dst]
    e_tile = tile_of[dst]
    e_slot = slot_of[dst]

    view0 = _build_view(cfg, [(0, NT)], core_of, tile_of, slot_of, src, dst,
                        e_core, e_tile, e_slot)
    view1 = _build_view(cfg, c["CHUNKS"], core_of, tile_of, slot_of, src, dst,
                        e_core, e_tile, e_slot)

    gsel = np.zeros((NC, 128, NT * G), np.float32)
    nid = np.arange(N)
    gsel[core_of, slot_of, tile_of * G + graph_ids[nid]] = 1.0

    x_fm = np.zeros((NC, F_IN, NPC), np.float16)
    colv = tile_of * 128 + slot_of
    for co in range(NC):
        m = core_of == co
        x_fm[co][:, colv[m]] = x[m].T.astype(np.float16)

    rep = lambda v, p=128: np.broadcast_to(
        np.asarray(v, np.float16)[None, :], (p, len(v))
    ).copy()

    def w16(k):
        return np.asarray(inputs[k]).astype(np.float16)

    def ws_pad(k):
        w = np.asarray(inputs[k]).astype(np.float16)
        out = np.zeros((w.shape[0], TE), np.float16)
        out[:, :HD] = w
        return out

    a_flat = [np.asarray(inputs[f"a{l}"]).astype(np.float32).reshape(-1) for l in (1, 2, 3)]
    b_flat = [np.asarray(inputs[f"b{l}"]).astype(np.float32) for l in (1, 2, 3)]
    b3m = b_flat[2].reshape(c["H"], c["D"]).mean(0)
    ECMAX = max(max(view0["EC"]), max(view1["EC"]))

    bex = np.asarray(inputs["bex"]).astype(np.float32)
    bex96 = np.concatenate([bex, bex, bex])

    common = dict(
        W1s=ws_pad("W1s"), W1d=w16("W1d"),
        W2s=ws_pad("W2s"), W2d=w16("W2d"),
        W3s=ws_pad("W3s"), W3d=w16("W3d"),
        a1_rep=rep(np.tile(a_flat[0], ECMAX)),
        a2_rep=rep(np.tile(a_flat[1], ECMAX)),
        a3_rep=rep(np.tile(a_flat[2], ECMAX)),
        b1_rep=rep(b_flat[0]), b2_rep=rep(b_flat[1]),
        b3m_rep=rep(b3m),
        ident8=np.eye(128, dtype=np.float32).astype(FP8NP),
        ident=np.eye(128, dtype=np.float16),
        p1T=w16("p1").T.copy(), p2T=w16("p2").T.copy(), p3T=w16("p3").T.copy(),
        Wex=w16("Wex"), bex96_rep=rep(bex96, G),
        Wpat=w16("Wpat"), bpat_rep=rep(np.asarray(inputs["bpat"], np.float32), G),
        Wc1=w16("Wc1"), bc1_rep=rep(np.asarray(inputs["bc1"], np.float32), G),
        Wc2=w16("Wc2"), bc2_rep=rep(np.asarray(inputs["bc2"], np.float32), G),
        Wc3=w16("Wc3"), bc3_rep=rep(np.asarray(inputs["bc3"], np.float32), G),
    )

    in_maps = []
    for co in range(NC):
        m = dict(common)
        m["x_fm"] = x_fm[co]
        m["gsel_all"] = gsel[co].astype(FP8NP)
        for vi, v in ((0, view0), (1, view1)):
            m[f"idxA{vi}"] = v["idxA"][co]
            m[f"idxB{vi}"] = v["idxB"][co]
            m[f"snm{vi}"] = v["snm"][co].astype(FP8NP)
            m[f"st{vi}"] = v["stt"][co].astype(FP8NP)
        in_maps.append(m)

    meta = dict(views=[
        {k: v[k] for k in ("eca", "ecb", "EC", "offA", "offB", "offO",
                           "FA_TOT", "FB_TOT", "O_TOT", "chunk_base",
                           "chunk_rows")}
        for v in (view0, view1)
    ])
    return in_maps, meta


# ---------------------------------------------------------------- device build

def build_gat(cfg, meta):
    c = cfg
    NC, NPC, NPAD, NT, G = c["NC"], c["NPC"], c["NPAD"], c["NT"], c["G"]
    H, D, HD, F_IN, TE, SPLIT = c["H"], c["D"], c["HD"], c["F_IN"], c["TE"], c["SPLIT"]
    HPAD = c["HPAD"]
    CHUNKS = c["CHUNKS"]
    V = meta["views"]
    ECMAX = max(max(V[0]["EC"]), max(V[1]["EC"]))
    view_of = {1: 0, 2: 1, 3: 1}

    nc = bacc.Bacc("TRN2", target_bir_lowering=False, debug=False, num_devices=NC,
                   num_swdge_queues=2)

    def din(name, shape, dt=F16):
        return nc.dram_tensor(name, shape, dt, kind="ExternalInput")

    x_fm = din("x_fm", [F_IN, NPC])
    gsel_all = din("gsel_all", [128, NT * G], F8)
    idx_d = {}
    oh_d = {}
    for vi in (0, 1):
        idx_d[vi] = (
            din(f"idxA{vi}", [128, V[vi]["FA_TOT"]], I16),
            din(f"idxB{vi}", [128, V[vi]["FB_TOT"]], I16),
        )
        oh_d[vi] = (
            din(f"snm{vi}", [128, V[vi]["O_TOT"]], F8),
            din(f"st{vi}", [128, V[vi]["O_TOT"]], F8),
        )

    Wmat = {
        1: (din("W1s", [F_IN, TE]), din("W1d", [F_IN, HD])),
        2: (din("W2s", [HD, TE]), din("W2d", [HD, HD])),
        3: (din("W3s", [HD, TE]), din("W3d", [HD, HD])),
    }
    a_rep = {l: din(f"a{l}_rep", [128, ECMAX * HD]) for l in (1, 2, 3)}
    b_rep = {1: din("b1_rep", [128, HD]), 2: din("b2_rep", [128, HD])}
    b3m_rep = din("b3m_rep", [128, D])
    ident8_d = din("ident8", [128, 128], F8)
    ident_d = din("ident", [128, 128])
    p123T = [din("p1T", [64, G]), din("p2T", [64, G]), din("p3T", [64, G])]
    Wex = din("Wex", [64, 32])
    bex96_rep = din("bex96_rep", [G, 96])
    Wpat = din("Wpat", [96, 64])
    bpat_rep = din("bpat_rep", [G, 64])
    Wc1 = din("Wc1", [128, 64])
    bc1_rep = din("bc1_rep", [G, 64])
    Wc2 = din("Wc2", [64, 32])
    bc2_rep = din("bc2_rep", [G, 32])
    Wc3 = din("Wc3", [32, 2])
    bc3_rep = din("bc3_rep", [G, 2])

    out = nc.dram_tensor("out", [G, 2], F32, kind="ExternalOutput")

    # internal DRAM
    fs_own1 = nc.dram_tensor("fs_own1", [NPC, TE], F16)
    fs_own = {
        (l, ci): nc.dram_tensor(f"fs_own{l}_{ci}", [rows, TE], F16)
        for l in (2, 3)
        for ci, rows in enumerate(V[1]["chunk_rows"])
    }
    h_dram = {
        (l, ci): nc.dram_tensor(f"h{l}_{ci}", [V[1]["chunk_rows"][ci], HPAD], F16)
        for l in (1, 2)
        for ci in range(len(CHUNKS))
    }
    fs_full = {
        l: nc.dram_tensor(f"fs_full{l}", [NPAD, TE], F16, addr_space="Shared")
        for l in (1, 2, 3)
    }
    partials = nc.dram_tensor("partials", [G, 65], F32)
    partials_red = nc.dram_tensor("partials_red", [G, 65], F32, addr_space="Shared")

    groups = [list(range(NC))]
    FCH = [(0, 128), (128, 64)]

    with tile.TileContext(nc) as tc:
        with (
            tc.tile_pool(name="const", bufs=1) as cpool,
            tc.tile_pool(name="wpool", bufs=1) as wpool,
            tc.tile_pool(name="hT", bufs=2) as hTpool,
            tc.tile_pool(name="proj", bufs=3) as ppool,
            tc.tile_pool(name="edge", bufs=2) as epool,
            tc.tile_pool(name="gath", bufs=3) as gpool,
            tc.tile_pool(name="oneh", bufs=3) as opool,
            tc.tile_pool(name="small", bufs=2) as spool,
            tc.tile_pool(name="psA", bufs=2, space="PSUM") as psA,
            tc.tile_pool(name="psZ", bufs=2, space="PSUM") as psZ,
            tc.tile_pool(name="psB", bufs=2, space="PSUM") as psB,
            tc.tile_pool(name="psT", bufs=1, space="PSUM") as psT,
            tc.tile_pool(name="psG", bufs=1, space="PSUM") as psG,
        ):
            nc.gpsimd.load_library(library_config.mlp)

            # ---------- resident constants
            ident8_t = cpool.tile([128, 128], F8)
            nc.sync.dma_start(ident8_t[:], ident8_d[:])
            ident_t = cpool.tile([128, 128], F16)
            nc.sync.dma_start(ident_t[:], ident_d[:])
            a_t = {l: cpool.tile([128, ECMAX * HD], F16, tag=f"a{l}", name=f"a{l}_t") for l in (1, 2, 3)}
            for l in (1, 2, 3):
                nc.sync.dma_start(a_t[l][:], a_rep[l][:])
            b_t = {l: cpool.tile([128, HD], F16, tag=f"b{l}", name=f"b{l}_t") for l in (1, 2)}
            for l in (1, 2):
                nc.sync.dma_start(b_t[l][:], b_rep[l][:])
            b3m_t = cpool.tile([128, D], F16)
            nc.sync.dma_start(b3m_t[:], b3m_rep[:])
            x_fm_t = cpool.tile([F_IN, NPC], F16)
            nc.sync.dma_start(x_fm_t[:], x_fm[:])
            gsel_t = cpool.tile([128, NT * G], F8)
            nc.sync.dma_start(gsel_t[:], gsel_all[:])
            idx_t = {}
            for vi in (0, 1):
                ta = cpool.tile([128, V[vi]["FA_TOT"]], I16, tag=f"ixA{vi}", name=f"idxA{vi}_t")
                nc.sync.dma_start(ta[:], idx_d[vi][0][:])
                tb = cpool.tile([128, V[vi]["FB_TOT"]], I16, tag=f"ixB{vi}", name=f"idxB{vi}_t")
                nc.sync.dma_start(tb[:], idx_d[vi][1][:])
                idx_t[vi] = (ta, tb)

            Wt = {}
            for l in (1, 2, 3):
                kdim = F_IN if l == 1 else HD
                chs = [(0, kdim)] if kdim <= 128 else FCH
                Wt[l] = []
                for k, (off, sz) in enumerate(chs):
                    ws = wpool.tile([sz, TE], F16, tag=f"W{l}s{k}", name=f"W{l}s{k}_t")
                    wd = wpool.tile([sz, HD], F16, tag=f"W{l}d{k}", name=f"W{l}d{k}_t")
                    nc.sync.dma_start(ws[:], Wmat[l][0][off : off + sz, :])
                    nc.sync.dma_start(wd[:], Wmat[l][1][off : off + sz, :])
                    Wt[l].append((ws, wd))

            fd_res = [
                cpool.tile([128, NT, HD], F16, tag=f"fd{i}", name=f"fd_res{i}")
                for i in (0, 1)
            ]
            fd_of = {1: fd_res[0], 2: fd_res[1], 3: fd_res[0]}

            gp_ps = psG.tile([G, 65], F32, space="PSUM")

            # ---------- pattern branch early (input-only)
            px_ps = psA.tile([G, 96], F32, space="PSUM", tag="psP", name="px_ps")
            Wex_t = spool.tile([64, 32], F16, tag="Wex_t")
            nc.sync.dma_start(Wex_t[:], Wex[:])
            for i in range(3):
                pT = spool.tile([64, G], F16, tag=f"pT{i}", name=f"pT{i}")
                nc.sync.dma_start(pT[:], p123T[i][:])
                nc.tensor.matmul(
                    px_ps[:, 32 * i : 32 * i + 32], lhsT=pT[:], rhs=Wex_t[:],
                    start=True, stop=True,
                )
            bex_t = spool.tile([G, 96], F16, tag="bex_t")
            nc.sync.dma_start(bex_t[:], bex96_rep[:])
            pxc = spool.tile([G, 96], F16, tag="pxc")
            nc.vector.tensor_tensor(
                out=pxc[:], in0=px_ps[:], in1=bex_t[:], op=mybir.AluOpType.add
            )

            # ---------- helpers
            def proj_tile(l, lhs_chunks, t, fs_dst, row0):
                """Project tile t for layer l into fs_dst rows [row0:row0+128]
                and fd_of[l][:, t, :]."""
                ps_fs = psA.tile([128, TE], F32, space="PSUM", tag="psP", name="ps_fs")
                ps_fd = psA.tile([128, HD], F32, space="PSUM", tag="psP", name="ps_fd")
                for k, lt in enumerate(lhs_chunks):
                    nc.tensor.matmul(
                        ps_fs[:], lhsT=lt, rhs=Wt[l][k][0][:],
                        start=(k == 0), stop=(k == len(lhs_chunks) - 1),
                    )
                for k, lt in enumerate(lhs_chunks):
                    nc.tensor.matmul(
                        ps_fd[:], lhsT=lt, rhs=Wt[l][k][1][:],
                        start=(k == 0), stop=(k == len(lhs_chunks) - 1),
                    )
                fs_sb = ppool.tile([128, TE], F16, tag="fs_sb")
                nc.scalar.copy(fs_sb[:], ps_fs[:])
                nc.scalar.copy(fd_of[l][:, t, :], ps_fd[:])
                nc.sync.dma_start(fs_dst[row0 : row0 + 128, :], fs_sb[:])

            def ag_full_l1():
                nc.gpsimd.collective_compute(
                    "AllGather",
                    mybir.AluOpType.bypass,
                    replica_groups=groups,
                    ins=[fs_own1[:].rearrange("a b -> (a b)")],
                    outs=[fs_full[1][:].rearrange("a b -> (a b)")],
                )

            def ag_chunk(l, ci):
                rows = V[1]["chunk_rows"][ci]
                base = V[1]["chunk_base"][ci]
                nc.gpsimd.collective_compute(
                    "AllGather",
                    mybir.AluOpType.bypass,
                    replica_groups=groups,
                    ins=[fs_own[(l, ci)][:].rearrange("a b -> (a b)")],
                    outs=[
                        fs_full[l][base : base + NC * rows, :].rearrange(
                            "a b -> (a b)"
                        )
                    ],
                )

            # ---------- layer-1 projection prologue: all tiles, one AG
            for t in range(NT):
                proj_tile(1, [x_fm_t[:, bass.ts(t, 128)]], t, fs_own1, t * 128)
            ag_full_l1()

            # ---------- edge loop state (group-batched gathers, per-tile stages)
            GROUPS = [(s0, min(s0 + GSZ, NT)) for s0 in range(0, NT, GSZ)]
            group_of = {}
            for gi, (gs, ge) in enumerate(GROUPS):
                for t in range(gs, ge):
                    group_of[t] = gi
            ECGMAX = 0
            for vi in (0, 1):
                for gs, ge in GROUPS:
                    ECGMAX = max(ECGMAX, sum(V[vi]["EC"][t] for t in range(gs, ge)))

            def prefetch_group(l, gi):
                vi = view_of[l]
                v = V[vi]
                gs, ge = GROUPS[gi]
                eaG = sum(v["eca"][t] for t in range(gs, ge))
                ebG = sum(v["ecb"][t] for t in range(gs, ge))
                A = gpool.tile([128, ECGMAX, TE], F16, tag="A")
                nc.gpsimd.dma_gather(
                    out_ap=A[:, :eaG, :],
                    in_ap=fs_full[l][:SPLIT, :],
                    idxs_ap=idx_t[vi][0][:, v["offA"][gs] : v["offA"][ge]],
                    num_idxs=eaG * 128,
                    num_idxs_reg=eaG * 128,
                    elem_size=TE,
                    queue_num=0,
                    single_packet=False,
                )
                if ebG:
                    nc.gpsimd.dma_gather(
                        out_ap=A[:, eaG : eaG + ebG, :],
                        in_ap=fs_full[l][SPLIT:, :],
                        idxs_ap=idx_t[vi][1][:, v["offB"][gs] : v["offB"][ge]],
                        num_idxs=ebG * 128,
                        num_idxs_reg=ebG * 128,
                        elem_size=TE,
                        queue_num=1,
                        single_packet=False,
                    )
                ETG = v["offO"][ge] - v["offO"][gs]
                snm_g = opool.tile([128, ECGMAX * 128], F8, tag="snm")
                nc.sync.dma_start(
                    snm_g[:, :ETG], oh_d[vi][0][:, v["offO"][gs] : v["offO"][ge]]
                )
                st_g = opool.tile([128, ECGMAX * 128], F8, tag="st")
                nc.sync.dma_start(
                    st_g[:, :ETG], oh_d[vi][1][:, v["offO"][gs] : v["offO"][ge]]
                )
                return dict(A=A, snm=snm_g, st=st_g, eaG=eaG)

            def tile_maps(l, t):
                """(gmap, ohch, ea, eb): group-buffer chunk index per tile
                chunk, one-hot chunk base, per-section chunk counts."""
                vi = view_of[l]
                v = V[vi]
                gi = group_of[t]
                gs, ge = GROUPS[gi]
                eaG = sum(v["eca"][tt] for tt in range(gs, ge))
                preA = sum(v["eca"][tt] for tt in range(gs, t))
                preB = sum(v["ecb"][tt] for tt in range(gs, t))
                ea, eb = v["eca"][t], v["ecb"][t]
                gmap = [preA + j for j in range(ea)] + [
                    eaG + preB + j for j in range(eb)
                ]
                ohch = (v["offO"][t] - v["offO"][gs]) // 128
                return gmap, ohch, ea, eb

            def stage1(l, t, s):
                """z = fd[dst] + fs in PSUM chunk-pairs; leaky -> C."""
                ec, A, snm_g = s["ec"], s["A"], s["snm"]
                gmap, ohch = s["gmap"], s["ohch"]
                C = epool.tile([128, ECMAX, HD], F16, tag="C")
                for j0 in range(0, ec, 2):
                    jn = min(2, ec - j0)
                    zps = psZ.tile([128, 2, HD], F32, space="PSUM", tag="zps", name="zps")
                    for j in range(j0, j0 + jn):
                        nc.tensor.matmul(
                            zps[:, j - j0, :],
                            lhsT=snm_g[:, bass.ts(ohch + j, 128)],
                            rhs=fd_of[l][:, t, :],
                            start=True, stop=False,
                        )
                        nc.tensor.matmul(
                            zps[:, j - j0, :],
                            lhsT=ident8_t[:],
                            rhs=A[:, gmap[j], :HD],
                            start=False, stop=True,
                        )
                    nc.scalar.activation(
                        C[:, j0 : j0 + jn, :],
                        zps[:, :jn, :],
                        mybir.ActivationFunctionType.Prelu,
                        alpha=NEG_GAT,
                    )
                s["C"] = C

            def stage2(l, t, s, ci):
                ec, A, st_g, C = s["ec"], s["A"], s["st"], s["C"]
                gmap, ohch, ea, eb = s["gmap"], s["ohch"], s["ea"], s["eb"]
                AM = epool.tile([128, ECMAX, HD], F16, tag="AM")
                nc.vector.tensor_tensor(
                    out=AM[:, :ec, :], in0=C[:, :ec, :],
                    in1=a_t[l][:, : ec * HD].rearrange("p (a q) -> p a q", q=HD),
                    op=mybir.AluOpType.mult,
                )
                logit = spool.tile([128, ECMAX, H], F32, tag="logit")
                nc.vector.tensor_reduce(
                    out=logit[:, :ec, :],
                    in_=AM[:, :ec, :].rearrange("p a (h d) -> p a h d", h=H),
                    axis=mybir.AxisListType.X,
                    op=mybir.AluOpType.add,
                )
                EFX = epool.tile([128, ECMAX, HD + H], F16, tag="EFX")
                ex = EFX[:, :ec, HD : HD + H]
                nc.scalar.activation(
                    ex, logit[:, :ec, :], mybir.ActivationFunctionType.Exp
                )
                for c0, clen, g0 in ((0, ea, gmap[0]), (ea, eb, gmap[ea] if eb else 0)):
                    if clen:
                        nc.vector.tensor_tensor(
                            out=EFX[:, c0 : c0 + clen, :HD].rearrange(
                                "p a (h d) -> p a h d", h=H
                            ),
                            in0=A[:, g0 : g0 + clen, :HD].rearrange(
                                "p a (h d) -> p a h d", h=H
                            ),
                            in1=ex[:, c0 : c0 + clen, :, None].to_broadcast(
                                [128, clen, H, D]
                            ),
                            op=mybir.AluOpType.mult,
                        )
                ps_ud = psB.tile([128, HD + H], F32, space="PSUM", tag="ps_ud", name="ps_ud")
                for j in range(ec):
                    nc.tensor.matmul(
                        ps_ud[:], lhsT=st_g[:, bass.ts(ohch + j, 128)],
                        rhs=EFX[:, j, :],
                        start=(j == 0), stop=(j == ec - 1),
                    )
                dmax = spool.tile([128, H], F32, tag="dmax")
                nc.vector.tensor_scalar_max(dmax[:], ps_ud[:, HD : HD + H], 1e-9)
                rden = spool.tile([128, H], F32, tag="rden")
                nc.vector.reciprocal(rden[:], dmax[:])
                hm = spool.tile([128, H, D], F16, tag="hm")
                nc.vector.tensor_tensor(
                    out=hm[:],
                    in0=ps_ud[:, :HD].rearrange("p (h d) -> p h d", h=H),
                    in1=rden[:, :, None].to_broadcast([128, H, D]),
                    op=mybir.AluOpType.mult,
                )
                if l < 3:
                    t0c = CHUNKS[ci][0]
                    h_sb = ppool.tile([128, HPAD], F16, tag="h_sb")
                    nc.vector.tensor_tensor(
                        out=h_sb[:, :HD],
                        in0=hm[:].rearrange("p h d -> p (h d)"),
                        in1=b_t[l][:],
                        op=mybir.AluOpType.add,
                    )
                    nc.vector.memset(h_sb[:, HD:], 0.0)
                    nc.sync.dma_start(
                        h_dram[(l, ci)][bass.ts(t - t0c, 128), :], h_sb[:]
                    )
                else:
                    rhs65 = ppool.tile([128, 65], F16, tag="rhs65")
                    t01 = spool.tile([128, D], F16, tag="t01")
                    nc.vector.tensor_tensor(
                        out=t01[:], in0=hm[:, 0, :], in1=hm[:, 1, :],
                        op=mybir.AluOpType.add,
                    )
                    t012 = spool.tile([128, D], F16, tag="t012")
                    nc.vector.tensor_tensor(
                        out=t012[:], in0=t01[:], in1=hm[:, 2, :],
                        op=mybir.AluOpType.add,
                    )
                    nc.vector.scalar_tensor_tensor(
                        out=rhs65[:, :D], in0=t012[:], scalar=1.0 / H,
                        in1=b3m_t[:], op0=mybir.AluOpType.mult,
                        op1=mybir.AluOpType.add,
                    )
                    nc.vector.memset(rhs65[:, 64:65], 1.0)
                    nc.tensor.matmul(
                        gp_ps[:], lhsT=gsel_t[:, bass.ts(t, G)], rhs=rhs65[:],
                        start=(t == 0), stop=(t == NT - 1),
                    )

            chunk_of = {}
            for ci, (t0, t1) in enumerate(CHUNKS):
                for t in range(t0, t1):
                    chunk_of[t] = ci
            chunk_last = {CHUNKS[ci][1] - 1: ci for ci in range(len(CHUNKS))}

            # ---------- layers: pipelined edge loop
            for l in (1, 2, 3):
                state = {}
                gstate = {}
                pending = deque()
                pending_ag = deque()

                def ensure_group(gi):
                    if gi < len(GROUPS) and gi not in gstate:
                        gstate[gi] = prefetch_group(l, gi)

                def drain_proj(n):
                    for _ in range(min(n, len(pending))):
                        ll, ci, tg, hT1, hT2 = pending.popleft()
                        tloc = tg - CHUNKS[ci][0]
                        proj_tile(
                            ll,
                            [hT1[:, bass.ts(tloc, 128)], hT2[:64, bass.ts(tloc, 128)]],
                            tg, fs_own[(ll, ci)], tloc * 128,
                        )
                    if not pending and pending_ag:
                        ll, ci = pending_ag.popleft()
                        ag_chunk(ll, ci)

                ensure_group(0)
                ensure_group(1)
                for p in range(NT + 2):
                    if p < NT:
                        gi = group_of[p]
                        if p == GROUPS[gi][0]:
                            ensure_group(gi + 1)
                        gmap, ohch, ea, eb = tile_maps(l, p)
                        g = gstate[gi]
                        state[p] = dict(
                            A=g["A"], snm=g["snm"], st=g["st"], ec=ea + eb,
                            gmap=gmap, ohch=ohch, ea=ea, eb=eb,
                        )
                    if 1 <= p <= NT:
                        stage1(l, p - 1, state[p - 1])
                    if p >= 2:
                        t2 = p - 2
                        stage2(l, t2, state[t2], chunk_of[t2])
                        del state[t2]["C"]
                        if l < 3 and t2 in chunk_last:
                            ci = chunk_last[t2]
                            rows = V[1]["chunk_rows"][ci]
                            hT1 = hTpool.tile([128, 2048], F16, tag="hT1", name="hT1")
                            nc.sync.dma_start(
                                hT1[:, :rows], h_dram[(l, ci)][:, 0:128],
                                transpose=True,
                            )
                            hT2 = hTpool.tile([128, 2048], F16, tag="hT2", name="hT2")
                            nc.sync.dma_start(
                                hT2[:, :rows], h_dram[(l, ci)][:, 128:256],
                                transpose=True,
                            )
                            for tg in range(CHUNKS[ci][0], CHUNKS[ci][1]):
                                pending.append((l + 1, ci, tg, hT1, hT2))
                            pending_ag.append((l + 1, ci))
                    drain_proj(4)
                while pending or pending_ag:
                    drain_proj(4)

            # ================= epilogue
            part_sb = spool.tile([G, 65], F32, tag="part_sb")
            nc.vector.tensor_copy(part_sb[:], gp_ps[:])
            nc.sync.dma_start(partials[:], part_sb[:])
            nc.gpsimd.collective_compute(
                "AllReduce",
                mybir.AluOpType.add,
                replica_groups=groups,
                ins=[partials[:]],
                outs=[partials_red[:]],
            )
            red_sb = spool.tile([G, 65], F32, tag="red_sb")
            nc.sync.dma_start(red_sb[:], partials_red[:])

            xg = spool.tile([G, 128], F16, tag="xg")
            rc = spool.tile([G, 1], F32, tag="rc")
            cnt1 = spool.tile([G, 1], F32, tag="cnt1")
            nc.vector.tensor_scalar_max(cnt1[:], red_sb[:, 64:65], 1.0)
            nc.vector.reciprocal(rc[:], cnt1[:])
            nc.vector.tensor_tensor(
                out=xg[:, :64], in0=red_sb[:, :64],
                in1=rc[:].to_broadcast([G, 64]), op=mybir.AluOpType.mult,
            )

            def small_mm(x_sb, pdim, w_t, b_t_, odim, leaky, out_ap, out_f32=False):
                tp = psT.tile([128, 128], F16, space="PSUM", tag="tp", name="ep_tp")
                nc.tensor.transpose(tp[:pdim, :G], x_sb[:, :pdim], ident_t[:G, :G])
                xT = spool.tile([128, G], F16, tag="ep_xT")
                nc.scalar.copy(xT[:pdim, :], tp[:pdim, :G])
                mm = psA.tile([G, 64], F32, space="PSUM", tag="psP", name="ep_mm")
                nc.tensor.matmul(
                    mm[:, :odim], lhsT=xT[:pdim, :], rhs=w_t[:], start=True, stop=True
                )
                tmp = spool.tile([G, 64], F32 if out_f32 else F16, tag="ep_tmp")
                nc.vector.tensor_tensor(
                    out=tmp[:, :odim], in0=mm[:, :odim], in1=b_t_[:],
                    op=mybir.AluOpType.add,
                )
                if leaky:
                    nc.vector.scalar_tensor_tensor(
                        out=out_ap, in0=tmp[:, :odim], scalar=NEG,
                        in1=tmp[:, :odim], op0=mybir.AluOpType.mult,
                        op1=mybir.AluOpType.max,
                    )
                else:
                    nc.vector.tensor_copy(out_ap, tmp[:, :odim])

            Wpat_t = spool.tile([96, 64], F16, tag="Wpat_t")
            nc.sync.dma_start(Wpat_t[:], Wpat[:])
            bpat_t = spool.tile([G, 64], F16, tag="bpat_t")
            nc.sync.dma_start(bpat_t[:], bpat_rep[:])
            small_mm(pxc, 96, Wpat_t, bpat_t, 64, True, xg[:, 64:128])

            Wc1_t = spool.tile([128, 64], F16, tag="Wc1_t")
            nc.sync.dma_start(Wc1_t[:], Wc1[:])
            bc1_t = spool.tile([G, 64], F16, tag="bc1_t")
            nc.sync.dma_start(bc1_t[:], bc1_rep[:])
            h1 = spool.tile([G, 64], F16, tag="ep_h1")
            small_mm(xg, 128, Wc1_t, bc1_t, 64, True, h1[:])

            Wc2_t = spool.tile([64, 32], F16, tag="Wc2_t")
            nc.sync.dma_start(Wc2_t[:], Wc2[:])
            bc2_t = spool.tile([G, 32], F16, tag="bc2_t")
            nc.sync.dma_start(bc2_t[:], bc2_rep[:])
            h2 = spool.tile([G, 32], F16, tag="ep_h2")
            small_mm(h1, 64, Wc2_t, bc2_t, 32, True, h2[:])

            Wc3_t = spool.tile([32, 2], F16, tag="Wc3_t")
            nc.sync.dma_start(Wc3_t[:], Wc3[:])
            bc3_t = spool.tile([G, 2], F16, tag="bc3_t")
            nc.sync.dma_start(bc3_t[:], bc3_rep[:])
            h3 = spool.tile([G, 2], F32, tag="ep_h3")
            small_mm(h2, 32, Wc3_t, bc3_t, 2, False, h3[:], out_f32=True)
            nc.sync.dma_start(out[:], h3[:])

    nc.finalize()
    return nc


# ---------------------------------------------------------------- entry point

def _run(inputs, trace=False, **trace_kwargs):
    cfg = _derive(_default_cfg())
    in_maps, meta = prep_host(inputs, cfg)
    nc = build_gat(cfg, meta)
    res = run_bass_kernel_spmd(
        nc, in_maps, core_ids=list(range(cfg["NC"])), trace=trace, **trace_kwargs
    )
    return np.asarray(res.results[0]["out"], np.float32), res


def kernel(**inputs):
    out, _ = _run(inputs, trace=False)
    return out
